# revision 8
# baseline (speedup 1.0000x reference)
"""GPTQ 4-bit linear kernel for Trainium2, 8-core token-parallel SPMD.

Math:  out[m,n] = sum_k x[m,k] * W[k,n],  W = scale[g,n] * (q[k,n] - z[g,n] - 1),
       g = k // 128 (group size 128 == SBUF partition count).

Decomposition: W = scale*q - scale*(z+1), so
    out = x @ (scale*q)  +  S @ zc
with S[m,g] = sum_{k in g} x[m,k] (computed on-device via one-hot matmuls) and
zc[g,n] = -(scale[g,n]*(z[g,n]+1)) (host-prepared quantization constants).

Per core (token shard m of 1024):
 - x shipped transposed+row-permuted as fp16 xtp[k', m]; within every
   256-row block, rows are reordered evens-then-odds so that one 128-row
   byte-tile of packed qweight aligns with one 128-partition weight tile.
 - qweight shipped as a byte-transposed uint8 array qb[k//2, n]; on-chip
   nibble extract (DVE) + fp16 scale multiply produce fp16 weight tiles.
 - TensorE: all matmuls use 256-wide moving slices, 4 consecutive moving
   chunks per stationary weight tile (measured 1.8x faster than single
   512-wide matmuls on TRN2 - weight loads hide behind short matmuls).
 - n processed in chunks of 512 columns (8 chunks), with the dequant
   pipeline double-buffered so DVE work overlaps the previous chunk's PE.
 - output written as out.T [n, m] fp16 tiles; host reassembles/transposes.
"""

import numpy as np

import concourse.bass as bass
import concourse.tile as tile
import concourse.mybir as mybir
from concourse import bacc
from concourse.bass_utils import run_bass_kernel_spmd

NCORES = 8
B, SEQ, IN_F, OUT_F = 4, 2048, 4096, 4096
GS = 128
NG = IN_F // GS          # 32 groups
NT_K = IN_F // 128       # 32 k' tiles
M_TOT = B * SEQ          # 8192 tokens
M = M_TOT // NCORES      # 1024 tokens per core
NCH = 512                # n columns per chunk (8 chunks)
MV = 256                 # moving-dim slice per matmul
F16 = mybir.dt.float16
F32 = mybir.dt.float32
F32R = mybir.dt.float32r
U8 = mybir.dt.uint8

_cache = {}


def _build(m=M, iters=1):
    nc = bacc.Bacc("TRN2", target_bir_lowering=False, debug=False,
                   num_devices=NCORES)
    xtp = nc.dram_tensor("xtp", [IN_F, m], F16, kind="ExternalInput").ap()
    qb = nc.dram_tensor("qb", [IN_F // 2, OUT_F], U8, kind="ExternalInput").ap()
    scl = nc.dram_tensor("scl", [NG, OUT_F], F16, kind="ExternalInput").ap()
    zc = nc.dram_tensor("zc", [NG, OUT_F], F32R, kind="ExternalInput").ap()
    gm = nc.dram_tensor("gm", [IN_F, NG], F16, kind="ExternalInput").ap()
    outT = nc.dram_tensor("outT", [OUT_F, m], F16, kind="ExternalOutput").ap()

    n_mh = m // 512       # m half-chunks of 512
    n_mv = m // MV        # 256-wide moving slices

    with tile.TileContext(nc) as tc:
        with tc.tile_pool(name="resident", bufs=1) as res:
            # resident activations: [128, t*m + m_local]
            xtp_sb = res.tile([128, NT_K * m], F16)
            for t in range(NT_K):
                nc.sync.dma_start(xtp_sb[:, bass.ts(t, m)],
                                  xtp[t * 128:(t + 1) * 128, :])
            gm_sb = res.tile([128, NT_K * NG], F16)
            for t in range(NT_K):
                nc.sync.dma_start(gm_sb[:, bass.ts(t, NG)],
                                  gm[t * 128:(t + 1) * 128, :])
            st_sb = res.tile([NG, m], F32R)

            # --- S phase: S.T[g, m] = sum_{k in g} xtp[k, m] via one-hot matmuls
            with tc.tile_pool(name="psS", bufs=1, space="PSUM") as psS_pool:
                psS = [psS_pool.tile([NG, 512], F32, tag=f"psS{i}", name=f"psS{i}")
                       for i in range(n_mh)]
                for t in range(NT_K):
                    for mc in range(n_mh):
                        nc.tensor.matmul(
                            psS[mc][:], gm_sb[:, bass.ts(t, NG)],
                            xtp_sb[:, bass.ds(t * m + mc * 512, 512)],
                            start=(t == 0), stop=(t == NT_K - 1))
                for mc in range(n_mh):
                    nc.scalar.copy(st_sb[:, bass.ts(mc, 512)], psS[mc][:])

            # --- main: per n-chunk of NCH columns
            from contextlib import ExitStack
            _loop = ExitStack()
            if iters > 1:
                _loop.enter_context(tc.For_i(0, iters, 1))
            with tc.tile_pool(name="wf", bufs=2) as wfp, \
                 tc.tile_pool(name="work", bufs=1) as work, \
                 tc.tile_pool(name="scp", bufs=1) as scp, \
                 tc.tile_pool(name="stage", bufs=2) as stage, \
                 tc.tile_pool(name="ps", bufs=1, space="PSUM") as psp:
                n_nt = NCH // 128
                for nci in range(OUT_F // NCH):
                    n0 = nci * NCH
                    # dequant: 16 byte-tiles -> 128 standalone fp16 weight
                    # tiles [128, 128] (full tiles keep fast weight load on PE)
                    wfs = [[None] * n_nt for _ in range(NT_K)]
                    nibs = []
                    for p in range(16):
                        qb_t = work.tile([128, NCH], U8, tag=f"qb{p}",
                                         name=f"qb_{nci}_{p}")
                        nc.sync.dma_start(
                            qb_t[:], qb[p * 128:(p + 1) * 128, n0:n0 + NCH])
                        sc_t = scp.tile([128, NCH], F16, tag=f"sc{p}",
                                        name=f"sc_{nci}_{p}")
                        for j in range(2):
                            nc.sync.dma_start(
                                sc_t[64 * j:64 * (j + 1), :],
                                scl[2 * p + j, n0:n0 + NCH].partition_broadcast(64))
                        nibs_p = []
                        for hi in range(2):
                            t = 2 * p + hi
                            nib = work.tile([128, NCH], U8,
                                            tag=f"nib{p}_{hi}",
                                            name=f"nib_{nci}_{p}_{hi}")
                            if hi == 0:
                                nc.vector.tensor_scalar(
                                    nib[:], qb_t[:], 0xF, None,
                                    op0=mybir.AluOpType.bitwise_and)
                            else:
                                nc.vector.tensor_scalar(
                                    nib[:], qb_t[:], 4, None,
                                    op0=mybir.AluOpType.logical_shift_right)
                            wt = wfp.tile([128, 128], F16, tag=f"wf{t}_0",
                                          name=f"wf_{nci}_{t}_0")
                            nc.vector.tensor_tensor(
                                wt[:], nib[:, bass.ts(0, 128)],
                                sc_t[:, bass.ts(0, 128)],
                                op=mybir.AluOpType.mult)
                            wfs[t][0] = wt
                            nibs_p.append(nib)
                        nibs.append((nibs_p, sc_t))
                    # phase B: remaining n-tiles, overlapping this chunk's PE
                    for nt in range(1, n_nt):
                        for p in range(16):
                            nibs_p, sc_t = nibs[p]
                            for hi in range(2):
                                t = 2 * p + hi
                                wt = wfp.tile([128, 128], F16,
                                              tag=f"wf{t}_{nt}",
                                              name=f"wf_{nci}_{t}_{nt}")
                                nc.vector.tensor_tensor(
                                    wt[:], nibs_p[hi][:, bass.ts(nt, 128)],
                                    sc_t[:, bass.ts(nt, 128)],
                                    op=mybir.AluOpType.mult)
                                wfs[t][nt] = wt

                    zc_st = scp.tile([NG, NCH], F32R, tag="zcst",
                                     name=f"zc_{nci}")
                    nc.sync.dma_start(zc_st[:], zc[:, n0:n0 + NCH])

                    # matmuls: per (n-tile, m-half): one psum tile, 2 quarter
                    # matmuls per weight load (measured-fastest PE pattern)
                    for nt in range(n_nt):
                        stg = stage.tile([128, m], F16, tag="stg",
                                         name=f"stg_{nci}_{nt}")
                        zc_ap = zc_st[:, bass.ts(nt, 128)]
                        for h in range(n_mh):
                            ps = psp.tile([128, 512], F32, tag=f"ps{2*nt+h}",
                                          name=f"ps_{nci}_{nt}_{h}")
                            # zc correction first, full-width: exactly one
                            # start=True (whole-bank psum zero) per tile
                            nc.tensor.matmul(
                                ps[:], zc_ap,
                                st_sb[:, bass.ds(h * 512, 512)],
                                start=True, stop=False)
                            for t in range(NT_K):
                                w_ap = wfs[t][nt][:]
                                for v in range(2):
                                    nc.tensor.matmul(
                                        ps[:, bass.ts(v, MV)], w_ap,
                                        xtp_sb[:, bass.ds(t * m + h * 512
                                                          + v * MV, MV)],
                                        start=False,
                                        stop=(t == NT_K - 1),
                                        skip_group_check=True)
                            nc.scalar.copy(stg[:, bass.ds(h * 512, 512)],
                                           ps[:])
                        # output store on the gpsimd DMA queue so next-chunk
                        # qb/sc loads (sync queue) don't wait behind it
                        nc.gpsimd.dma_start(
                            outT[n0 + nt * 128:n0 + (nt + 1) * 128, :],
                            stg[:])
            _loop.close()
    nc.compile()
    return nc


def _prep(x, qweight, qzeros, scales, m=M, ncores=NCORES):
    """Host-side layout marshaling -> per-core input maps."""
    # activations: transpose + evens-then-odds permutation within 256-blocks
    x2 = np.ascontiguousarray(x.reshape(M_TOT, IN_F))
    perm = np.empty(IN_F, dtype=np.int64)
    for t in range(NT_K):
        P, par = divmod(t, 2)
        perm[t * 128:(t + 1) * 128] = 256 * P + 2 * np.arange(128) + par
    xtp = np.ascontiguousarray(x2.T[perm]).astype(np.float16)  # [IN_F, M_TOT]

    # packed weights as byte rows: qb[k//2, n] = byte holding nibbles (2bk, 2bk+1)
    qb = np.ascontiguousarray(
        qweight.view(np.uint8).reshape(IN_F // 8, OUT_F, 4)
        .transpose(0, 2, 1).reshape(IN_F // 2, OUT_F))

    # zero-point correction constants zc[g,n] = -(scale*(z+1))
    u = qzeros.view(np.uint32)
    shifts = (4 * np.arange(8, dtype=np.uint32))[None, None, :]
    z = ((u[:, :, None] >> shifts) & np.uint32(0xF)).reshape(NG, OUT_F)
    # device computes x @ (fp16(scale)*q); cancel with the same fp16 scale
    scl16 = scales.astype(np.float16)
    zcv = np.ascontiguousarray((-(scl16.astype(np.float64)
                                  * (z.astype(np.float64) + 1.0))).astype(np.float32))

    # one-hot group map in permuted k' order: gm[k', g] = 1 if group(k') == g
    gmv = np.zeros((IN_F, NG), dtype=np.float16)
    rows = np.arange(IN_F)
    t_idx = rows // 128
    p_idx = rows % 128
    g_idx = 2 * (t_idx // 2) + p_idx // 64
    gmv[rows, g_idx] = 1.0

    in_maps = []
    for c in range(ncores):
        in_maps.append({
            "xtp": np.ascontiguousarray(xtp[:, c * m:(c + 1) * m]),
            "qb": qb, "scl": scl16,
            "zc": zcv, "gm": gmv,
        })
    return in_maps


def kernel(x, qweight, qzeros, scales):
    x = np.ascontiguousarray(np.asarray(x, dtype=np.float32))
    qweight = np.ascontiguousarray(np.asarray(qweight, dtype=np.int32))
    qzeros = np.ascontiguousarray(np.asarray(qzeros, dtype=np.int32))
    scales = np.ascontiguousarray(np.asarray(scales, dtype=np.float32))
    if "nc" not in _cache:
        _cache["nc"] = _build()
    nc = _cache["nc"]
    in_maps = _prep(x, qweight, qzeros, scales)
    results = run_bass_kernel_spmd(
        nc, in_maps, core_ids=list(range(NCORES))).results
    outs = [r["outT"] for r in results]              # each [OUT_F, M] fp16
    full = np.concatenate(outs, axis=1)              # [OUT_F, M_TOT]
    return np.ascontiguousarray(full.T.astype(np.float32)).reshape(B, SEQ, OUT_F)


# revision 9
# speedup vs baseline: 1.3341x; 1.3341x over previous
"""GPTQ 4-bit linear kernel for Trainium2, 8-core token-parallel SPMD.

Math:  out[m,n] = sum_k x[m,k] * W[k,n],  W = scale[g,n] * (q[k,n] - z[g,n] - 1),
       g = k // 128 (group size 128 == SBUF partition count).

Decomposition: W = scale*q - scale*(z+1), so
    out = x @ (scale*q)  +  S @ zc
with S[m,g] = sum_{k in g} x[m,k] (computed on-device via one-hot matmuls) and
zc[g,n] = -(scale[g,n]*(z[g,n]+1)) (host-prepared quantization constants).

Per core (token shard m of 1024):
 - x shipped transposed+row-permuted as fp16 xtp[k', m]; within every
   256-row block, rows are reordered evens-then-odds so that one 128-row
   byte-tile of packed qweight aligns with one 128-partition weight tile.
 - qweight shipped as a byte-transposed uint8 array qb[k//2, n]; on-chip
   nibble extract (DVE) + fp16 scale multiply produce fp16 weight tiles.
 - TensorE: all matmuls use 256-wide moving slices, 4 consecutive moving
   chunks per stationary weight tile (measured 1.8x faster than single
   512-wide matmuls on TRN2 - weight loads hide behind short matmuls).
 - n processed in chunks of 512 columns (8 chunks), with the dequant
   pipeline double-buffered so DVE work overlaps the previous chunk's PE.
 - output written as out.T [n, m] fp16 tiles; host reassembles/transposes.
"""

import numpy as np

import concourse.bass as bass
import concourse.tile as tile
import concourse.mybir as mybir
from concourse import bacc
from concourse.bass_utils import run_bass_kernel_spmd

NCORES = 8
B, SEQ, IN_F, OUT_F = 4, 2048, 4096, 4096
GS = 128
NG = IN_F // GS          # 32 groups
NT_K = IN_F // 128       # 32 k' tiles
M_TOT = B * SEQ          # 8192 tokens
M = M_TOT // NCORES      # 1024 tokens per core
NCH = 512                # n columns per chunk (8 chunks)
MV = 256                 # moving-dim slice per matmul
F16 = mybir.dt.float16
F32 = mybir.dt.float32
F32R = mybir.dt.float32r
U8 = mybir.dt.uint8

_cache = {}


def _build(m=M, iters=1):
    nc = bacc.Bacc("TRN2", target_bir_lowering=False, debug=False,
                   num_devices=NCORES)
    xtp = nc.dram_tensor("xtp", [IN_F, m], F16, kind="ExternalInput").ap()
    qb = nc.dram_tensor("qb", [IN_F // 2, OUT_F], U8, kind="ExternalInput").ap()
    scl = nc.dram_tensor("scl", [NG, OUT_F], F16, kind="ExternalInput").ap()
    zc = nc.dram_tensor("zc", [NG, OUT_F], F32R, kind="ExternalInput").ap()
    gm = nc.dram_tensor("gm", [IN_F, NG], F16, kind="ExternalInput").ap()
    outT = nc.dram_tensor("outT", [OUT_F, m], F16, kind="ExternalOutput").ap()

    n_mh = m // 512       # m half-chunks of 512
    n_mv = m // MV        # 256-wide moving slices

    with tile.TileContext(nc) as tc:
        with tc.tile_pool(name="resident", bufs=1) as res:
            # resident activations: [128, t*m + m_local]
            xtp_sb = res.tile([128, NT_K * m], F16)
            for t in range(NT_K):
                nc.sync.dma_start(xtp_sb[:, bass.ts(t, m)],
                                  xtp[t * 128:(t + 1) * 128, :])
            gm_sb = res.tile([128, NT_K * NG], F16)
            for t in range(NT_K):
                nc.sync.dma_start(gm_sb[:, bass.ts(t, NG)],
                                  gm[t * 128:(t + 1) * 128, :])
            zc_sb = res.tile([NG, OUT_F], F32R)
            nc.sync.dma_start(zc_sb[:], zc)
            st_sb = res.tile([NG, m], F32R)

            # --- S phase: S.T[g, m] = sum_{k in g} xtp[k, m] via one-hot matmuls
            with tc.tile_pool(name="psS", bufs=1, space="PSUM") as psS_pool:
                psS = [psS_pool.tile([NG, 512], F32, tag=f"psS{i}", name=f"psS{i}")
                       for i in range(n_mh)]
                for t in range(NT_K):
                    for mc in range(n_mh):
                        nc.tensor.matmul(
                            psS[mc][:], gm_sb[:, bass.ts(t, NG)],
                            xtp_sb[:, bass.ds(t * m + mc * 512, 512)],
                            start=(t == 0), stop=(t == NT_K - 1))
                for mc in range(n_mh):
                    nc.scalar.copy(st_sb[:, bass.ts(mc, 512)], psS[mc][:])

            # --- main: per n-chunk of NCH columns
            from contextlib import ExitStack
            _loop = ExitStack()
            if iters > 1:
                _loop.enter_context(tc.For_i(0, iters, 1))
            with tc.tile_pool(name="wf", bufs=2) as wfp, \
                 tc.tile_pool(name="work", bufs=2) as work, \
                 tc.tile_pool(name="scp", bufs=1) as scp, \
                 tc.tile_pool(name="stage", bufs=2) as stage, \
                 tc.tile_pool(name="ps", bufs=1, space="PSUM") as psp:
                n_nt = NCH // 128
                for nci in range(OUT_F // NCH):
                    n0 = nci * NCH
                    # dequant: 16 byte-tiles -> 128 standalone fp16 weight
                    # tiles [128, 128] (full tiles keep fast weight load on PE)
                    wfs = [[None] * n_nt for _ in range(NT_K)]
                    for p in range(16):
                        qb_t = work.tile([128, NCH], U8, tag=f"qb{p}",
                                         name=f"qb_{nci}_{p}")
                        nc.sync.dma_start(
                            qb_t[:], qb[p * 128:(p + 1) * 128, n0:n0 + NCH])
                        sc_t = scp.tile([128, NCH], F16, tag=f"sc{p}",
                                        name=f"sc_{nci}_{p}")
                        for j in range(2):
                            nc.sync.dma_start(
                                sc_t[64 * j:64 * (j + 1), :],
                                scl[2 * p + j, n0:n0 + NCH].partition_broadcast(64))
                        for hi in range(2):
                            t = 2 * p + hi
                            nib = work.tile([128, NCH], U8, tag=f"nib{hi}",
                                            name=f"nib_{nci}_{p}_{hi}")
                            if hi == 0:
                                nc.vector.tensor_scalar(
                                    nib[:], qb_t[:], 0xF, None,
                                    op0=mybir.AluOpType.bitwise_and)
                            else:
                                nc.vector.tensor_scalar(
                                    nib[:], qb_t[:], 4, None,
                                    op0=mybir.AluOpType.logical_shift_right)
                            for nt in range(n_nt):
                                wt = wfp.tile([128, 128], F16,
                                              tag=f"wf{t}_{nt}",
                                              name=f"wf_{nci}_{t}_{nt}")
                                nc.vector.tensor_tensor(
                                    wt[:], nib[:, bass.ts(nt, 128)],
                                    sc_t[:, bass.ts(nt, 128)],
                                    op=mybir.AluOpType.mult)
                                wfs[t][nt] = wt

                    # matmuls: per (n-tile, m-half): one psum tile, 2 quarter
                    # matmuls per weight load (measured-fastest PE pattern)
                    for nt in range(n_nt):
                        stg = stage.tile([128, m], F16, tag="stg",
                                         name=f"stg_{nci}_{nt}")
                        zc_ap = zc_sb[:, bass.ds(n0 + nt * 128, 128)]
                        for h in range(n_mh):
                            ps = psp.tile([128, 512], F32, tag=f"ps{2*nt+h}",
                                          name=f"ps_{nci}_{nt}_{h}")
                            # zc correction first, full-width: exactly one
                            # start=True (whole-bank psum zero) per tile
                            nc.tensor.matmul(
                                ps[:], zc_ap,
                                st_sb[:, bass.ds(h * 512, 512)],
                                start=True, stop=False)
                            for t in range(NT_K):
                                w_ap = wfs[t][nt][:]
                                for v in range(2):
                                    nc.tensor.matmul(
                                        ps[:, bass.ts(v, MV)], w_ap,
                                        xtp_sb[:, bass.ds(t * m + h * 512
                                                          + v * MV, MV)],
                                        start=False,
                                        stop=(t == NT_K - 1),
                                        skip_group_check=True)
                            nc.scalar.copy(stg[:, bass.ds(h * 512, 512)],
                                           ps[:])
                        # output store on the gpsimd DMA queue so next-chunk
                        # qb/sc loads (sync queue) don't wait behind it
                        nc.gpsimd.dma_start(
                            outT[n0 + nt * 128:n0 + (nt + 1) * 128, :],
                            stg[:])
            _loop.close()
    nc.compile()
    return nc


def _prep(x, qweight, qzeros, scales, m=M, ncores=NCORES):
    """Host-side layout marshaling -> per-core input maps."""
    # activations: transpose + evens-then-odds permutation within 256-blocks
    x2 = np.ascontiguousarray(x.reshape(M_TOT, IN_F))
    perm = np.empty(IN_F, dtype=np.int64)
    for t in range(NT_K):
        P, par = divmod(t, 2)
        perm[t * 128:(t + 1) * 128] = 256 * P + 2 * np.arange(128) + par
    xtp = np.ascontiguousarray(x2.T[perm]).astype(np.float16)  # [IN_F, M_TOT]

    # packed weights as byte rows: qb[k//2, n] = byte holding nibbles (2bk, 2bk+1)
    qb = np.ascontiguousarray(
        qweight.view(np.uint8).reshape(IN_F // 8, OUT_F, 4)
        .transpose(0, 2, 1).reshape(IN_F // 2, OUT_F))

    # zero-point correction constants zc[g,n] = -(scale*(z+1))
    u = qzeros.view(np.uint32)
    shifts = (4 * np.arange(8, dtype=np.uint32))[None, None, :]
    z = ((u[:, :, None] >> shifts) & np.uint32(0xF)).reshape(NG, OUT_F)
    # device computes x @ (fp16(scale)*q); cancel with the same fp16 scale
    scl16 = scales.astype(np.float16)
    zcv = np.ascontiguousarray((-(scl16.astype(np.float64)
                                  * (z.astype(np.float64) + 1.0))).astype(np.float32))

    # one-hot group map in permuted k' order: gm[k', g] = 1 if group(k') == g
    gmv = np.zeros((IN_F, NG), dtype=np.float16)
    rows = np.arange(IN_F)
    t_idx = rows // 128
    p_idx = rows % 128
    g_idx = 2 * (t_idx // 2) + p_idx // 64
    gmv[rows, g_idx] = 1.0

    in_maps = []
    for c in range(ncores):
        in_maps.append({
            "xtp": np.ascontiguousarray(xtp[:, c * m:(c + 1) * m]),
            "qb": qb, "scl": scl16,
            "zc": zcv, "gm": gmv,
        })
    return in_maps


def kernel(x, qweight, qzeros, scales):
    x = np.ascontiguousarray(np.asarray(x, dtype=np.float32))
    qweight = np.ascontiguousarray(np.asarray(qweight, dtype=np.int32))
    qzeros = np.ascontiguousarray(np.asarray(qzeros, dtype=np.int32))
    scales = np.ascontiguousarray(np.asarray(scales, dtype=np.float32))
    if "nc" not in _cache:
        _cache["nc"] = _build()
    nc = _cache["nc"]
    in_maps = _prep(x, qweight, qzeros, scales)
    results = run_bass_kernel_spmd(
        nc, in_maps, core_ids=list(range(NCORES))).results
    outs = [r["outT"] for r in results]              # each [OUT_F, M] fp16
    full = np.concatenate(outs, axis=1)              # [OUT_F, M_TOT]
    return np.ascontiguousarray(full.T.astype(np.float32)).reshape(B, SEQ, OUT_F)
